# revision 1
# baseline (speedup 1.0000x reference)
"""Multi-head attention (RoPE-full-dmodel variant) on 8 TRN2 NeuronCores.

Sharding: core c = (batch c//4, head-group c%4 of 4 heads).
 - W_q/W_k/W_v split column-wise by head (each core projects its 256 channels)
 - W_o split row-wise; per-core partial outputs summed on host (all-reduce at gather)

Per-core kernel (matmul data fp16, accumulation/softmax stats f32):
  RoPE(q,k) on DVE/GpSimd -> Q^T/K^T/V^T projections (chan-major) -> V seq-major
  via PE transpose (with a ones-column so the PV matmul also produces softmax
  denominators) -> scoresT = K_h^T-stationary x Q_h^T (K=64) -> exp on ACT over
  qblock-pairs (scale=1/8 folded in; no max-subtraction: scores ~ N(0,1)) ->
  U^T = (V|1)-stationary x expS^T -> normalize via reciprocal row-sums ->
  output projection interleaved per qblock-pair (row-parallel partial).

Layout trick: activations are host-transposed to D-major with an even/odd row
permutation of the d_model axis so the interleaved-repeat RoPE tables collapse
to 512 distinct rows, partition-aligned in 128-chunks.
"""
import os
import sys
from contextlib import nullcontext

for _p in ("/opt/trn_rl_repo", "/root/.axon_site/_ro/trn_rl_repo"):
    if os.path.isdir(_p) and _p not in sys.path:
        sys.path.insert(0, _p)

import numpy as np

import concourse.bacc as bacc
import concourse.tile as tile
import concourse.mybir as mybir
from concourse.bass_utils import run_bass_kernel_spmd

B, S, D = 2, 2048, 1024
H_TOT, DK = 16, 64
N_CORES, GROUPS = 8, 4
CH = 256            # channels (heads*dk) per core
KC = D // 128       # 8 d-model chunks
QB = S // 512       # 4 query blocks
ST = S // 128       # 16 seq tiles
HPC = 4             # heads per core
BASE = 10000.0

MM = mybir.dt.float16
F32 = mybir.dt.float32
AF = mybir.ActivationFunctionType

# even/odd permutation of the d_model axis: row r <- old d = 2r (r<512), 2(r-512)+1
_PERM = np.concatenate([np.arange(0, D, 2), np.arange(1, D, 2)])

_PROG = None


def _build(loop_n=1, phases=("v", "rope", "proj", "attn", "ut", "out")):
    nc = bacc.Bacc("TRN2", target_bir_lowering=False, debug=False)
    qT = nc.dram_tensor("qT", (D, S), MM, kind="ExternalInput").ap()
    kT = nc.dram_tensor("kT", (D, S), MM, kind="ExternalInput").ap()
    vT = nc.dram_tensor("vT", (D, S), MM, kind="ExternalInput").ap()
    cosc = nc.dram_tensor("cosc", (D // 2, S), MM, kind="ExternalInput").ap()
    sinc = nc.dram_tensor("sinc", (D // 2, S), MM, kind="ExternalInput").ap()
    wq = nc.dram_tensor("wq", (D, CH), MM, kind="ExternalInput").ap()
    wk = nc.dram_tensor("wk", (D, CH), MM, kind="ExternalInput").ap()
    wv = nc.dram_tensor("wv", (D, CH), MM, kind="ExternalInput").ap()
    wo = nc.dram_tensor("wo", (CH, D), MM, kind="ExternalInput").ap()
    bq = nc.dram_tensor("bq", (2, 128, 1), F32, kind="ExternalInput").ap()
    bk = nc.dram_tensor("bk", (2, 128, 1), F32, kind="ExternalInput").ap()
    bv = nc.dram_tensor("bv", (HPC, 64, 1), F32, kind="ExternalInput").ap()
    ident = nc.dram_tensor("ident", (128, 128), MM, kind="ExternalInput").ap()
    out = nc.dram_tensor("out", (S, D), MM, kind="ExternalOutput").ap()

    with tile.TileContext(nc) as tc:
      with (
          tc.tile_pool(name="consts", bufs=1) as consts,
          tc.tile_pool(name="qkv", bufs=1) as qkv,
          tc.tile_pool(name="misc", bufs=3) as misc,
          tc.tile_pool(name="outst", bufs=4) as outst,
          tc.tile_pool(name="ps_gen", bufs=2, space="PSUM") as ps_gen,
          tc.tile_pool(name="ps_sc", bufs=2, space="PSUM") as ps_sc,
          tc.tile_pool(name="ps_ut", bufs=2, space="PSUM") as ps_ut,
      ):
        with (tc.For_i(0, loop_n, 1, hint_engines=tuple(mybir.ALL_ENGINES))
              if loop_n > 1 else nullcontext()):
            # ---- small constants ----
            wq_sb = consts.tile([128, KC * CH], MM, tag="wq")
            wk_sb = consts.tile([128, KC * CH], MM, tag="wk")
            wv_sb = consts.tile([128, KC * CH], MM, tag="wv")
            ident_sb = consts.tile([128, 128], MM, tag="ident")
            for c in range(KC):
                nc.sync.dma_start(wv_sb[:, CH * c:CH * (c + 1)], wv[128 * c:128 * (c + 1), :])
            nc.sync.dma_start(ident_sb[:], ident)
            wo_sb = consts.tile([128, 2 * D], MM, tag="wo")
            bq_sb, bk_sb, bv_sb = [], [], []
            for c in range(2):
                t_ = consts.tile([128, 1], F32, tag=f"bq{c}", name=f"bq{c}")
                nc.sync.dma_start(t_[:], bq[c])
                bq_sb.append(t_)
                t_ = consts.tile([128, 1], F32, tag=f"bk{c}", name=f"bk{c}")
                nc.sync.dma_start(t_[:], bk[c])
                bk_sb.append(t_)
            for h in range(HPC):
                t_ = consts.tile([64, 1], F32, tag=f"bv{h}", name=f"bv{h}")
                nc.sync.dma_start(t_[:], bv[h])
                bv_sb.append(t_)

            # V storage: per (tile t, head h) block of 65 cols: 64 values + ones col
            v_sb = qkv.tile([128, ST * HPC * 65], MM, tag="v")
            ones_cols = v_sb[:].rearrange("p (b c) -> p b c", c=65)[:, :, 64]
            nc.vector.memset(ones_cols, 1.0)

            qt_sb = [qkv.tile([128, S], MM, tag=f"qt{c}", name=f"qt{c}") for c in range(2)]
            kt_sb = [qkv.tile([128, S], MM, tag=f"kt{c}", name=f"kt{c}") for c in range(2)]
            ut_sb = [qkv.tile([128, S], MM, tag=f"ut{c}", name=f"ut{c}") for c in range(2)]

            # ================= phase 1: RoPE + projections =================
            with (
                tc.tile_pool(name="p1", bufs=1) as p1,
            ):
                ps_proj = ps_gen
                # ---- rope-k inputs + tables first: they gate RoPE, the
                #      phase-1 critical path ----
                rope_pre = {}
                for a in (0, 2, 1, 3):
                    xa = p1.tile([128, S], MM, tag="ri", name=f"xk{a}", bufs=4)
                    nc.sync.dma_start(xa[:], kT[128 * a:128 * (a + 1), :])
                    rope_pre[a] = xa
                cos_sb, sin_sb = [None] * 4, [None] * 4
                for t in (0, 2, 1, 3):
                    ct = p1.tile([128, S], MM, tag=f"cos{t}", name=f"cos{t}")
                    nc.sync.dma_start(ct[:], cosc[128 * t:128 * (t + 1), :])
                    cos_sb[t] = ct
                    st_ = p1.tile([128, S], MM, tag=f"sin{t}", name=f"sin{t}")
                    nc.sync.dma_start(st_[:], sinc[128 * t:128 * (t + 1), :])
                    sin_sb[t] = st_

                # ---- V^T projection (chan-major) + PE transpose to seq-major ----
                vt_cm = [p1.tile([128, S], MM, tag=f"vtc{c}", name=f"vtc{c}", bufs=1)
                         for c in range(2)]
                if "v" in phases:
                    vchunks = []
                    for d in range(KC):
                        vc = p1.tile([128, S], MM, tag="roped", name=f"vch{d}", bufs=8)
                        nc.sync.dma_start(vc[:], vT[128 * d:128 * (d + 1), :])
                        vchunks.append(vc)
                    for c in range(2):
                        for sp in range(0, QB, 2):
                            psums = [ps_proj.tile([128, 512], F32, tag="proj", name="psum")
                                     for _ in range(2)]
                            for d in range(KC):
                                lhsT = wv_sb[:, CH * d + 128 * c: CH * d + 128 * (c + 1)]
                                for i in range(2):
                                    s_ = sp + i
                                    nc.tensor.matmul(psums[i][:], lhsT,
                                                     vchunks[d][:, 512 * s_:512 * (s_ + 1)],
                                                     start=(d == 0), stop=(d == KC - 1))
                            for i in range(2):
                                s_ = sp + i
                                nc.scalar.copy(vt_cm[c][:, 512 * s_:512 * (s_ + 1)], psums[i][:])
                    for t in range(ST):
                        pv = ps_gen.tile([128, CH], MM, tag="proj")
                        for c in range(2):
                            nc.tensor.transpose(pv[:, 128 * c:128 * (c + 1)],
                                                vt_cm[c][:, 128 * t:128 * (t + 1)],
                                                ident_sb[:])
                        dst = _v_scatter_ap(v_sb, t)
                        nc.scalar.copy(dst, pv[:])

                # ---- RoPE + projections: K first, then Q (q-proj split so
                #      attention qbp0 can start after qblocks 0-1 project) ----
                SPL = 1664  # DVE fp16 2x (245G/s) : GPSIMD 0.42x (64G/s)

                def _ew(op, out, in0, in1):
                    getattr(nc.vector, op)(out[:, :SPL], in0[:, :SPL], in1[:, :SPL])
                    getattr(nc.gpsimd, op)(out[:, SPL:], in0[:, SPL:], in1[:, SPL:])

                def _rope(src_t, pre=None):
                    roped = [None] * KC
                    for a in (0, 1, 4, 5):
                        b_ = a + 2
                        if pre and a in pre:
                            xa = pre.pop(a)
                        else:
                            xa = p1.tile([128, S], MM, tag="ri", name="xa", bufs=4)
                            nc.sync.dma_start(xa[:], src_t[128 * a:128 * (a + 1), :])
                        if pre and b_ in pre:
                            xb = pre.pop(b_)
                        else:
                            xb = p1.tile([128, S], MM, tag="ri", name="xb", bufs=4)
                            nc.sync.dma_start(xb[:], src_t[128 * b_:128 * (b_ + 1), :])
                        ca, cb = a % 4, b_ % 4
                        t1 = p1.tile([128, S], MM, tag="tmp", name="t1", bufs=3)
                        _ew("tensor_mul", t1, xa, cos_sb[ca])
                        t2 = p1.tile([128, S], MM, tag="tmp", name="t2", bufs=3)
                        _ew("tensor_mul", t2, xb, sin_sb[ca])
                        ra = p1.tile([128, S], MM, tag="roped", name="ra", bufs=8)
                        _ew("tensor_sub", ra, t1, t2)
                        t3 = p1.tile([128, S], MM, tag="tmp", name="t3", bufs=3)
                        _ew("tensor_mul", t3, xb, cos_sb[cb])
                        t4 = p1.tile([128, S], MM, tag="tmp", name="t4", bufs=3)
                        _ew("tensor_mul", t4, xa, sin_sb[cb])
                        rb = p1.tile([128, S], MM, tag="roped", name="rb", bufs=8)
                        _ew("tensor_add", rb, t3, t4)
                        roped[a], roped[b_] = ra, rb
                    return roped

                def _proj(roped, w_sb, b_sb, dst_tiles, s_list):
                    s_list = list(s_list)
                    for c in range(2):
                        for sp in range(0, len(s_list), 2):
                            pair = s_list[sp:sp + 2]
                            psums = [ps_proj.tile([128, 512], F32, tag="proj", name="psum")
                                     for _ in pair]
                            for d in range(KC):
                                lhsT = w_sb[:, CH * d + 128 * c: CH * d + 128 * (c + 1)]
                                for i, s_ in enumerate(pair):
                                    nc.tensor.matmul(psums[i][:], lhsT,
                                                     roped[d][:, 512 * s_:512 * (s_ + 1)],
                                                     start=(d == 0), stop=(d == KC - 1))
                            for i, s_ in enumerate(pair):
                                nc.scalar.activation(
                                    dst_tiles[c][:, 512 * s_:512 * (s_ + 1)], psums[i][:],
                                    AF.Identity, bias=b_sb[c][:])

                if "rope" in phases:
                    roped_k = _rope(kT, rope_pre)
                    for c in range(KC):
                        nc.sync.dma_start(wk_sb[:, CH * c:CH * (c + 1)],
                                          wk[128 * c:128 * (c + 1), :])
                        nc.sync.dma_start(wq_sb[:, CH * c:CH * (c + 1)],
                                          wq[128 * c:128 * (c + 1), :])
                    if "proj" in phases:
                        _proj(roped_k, wk_sb, bk_sb, kt_sb, range(QB))
                    roped_q = _rope(qT)
                    if "proj" in phases:
                        _proj(roped_q, wq_sb, bq_sb, qt_sb, range(QB))

            # ================= phase 2: attention + output projection =================
                for c in range(2):
                    nc.sync.dma_start(wo_sb[:, D * c:D * (c + 1)], wo[128 * c:128 * (c + 1), :])
                with (
                    tc.tile_pool(name="expp", bufs=10) as expp,
                ):
                    def _attention(qbp):
                        q_lo = 1024 * qbp
                        for h in range(HPC):
                            ct, po = h // 2, 64 * (h % 2)
                            qt_h = qt_sb[ct][po:po + 64, q_lo:q_lo + 1024]
                            kt_h = kt_sb[ct][po:po + 64, :]
                            puts = [ps_ut.tile([65, 512], F32, tag="ut", name=f"put{half}")
                                    for half in range(2)]
                            for t in range(ST):
                                psc = ps_sc.tile([128, 1024], F32, tag="sc", name="psc")
                                for half in range(2):
                                    nc.tensor.matmul(
                                        psc[:, 512 * half:512 * (half + 1)],
                                        kt_h[:, 128 * t:128 * (t + 1)],
                                        qt_h[:, 512 * half:512 * (half + 1)],
                                        start=True, stop=True)
                                e = expp.tile([128, 1024], MM, tag="e", name="e")
                                nc.scalar.activation(e[:], psc[:], AF.Exp, scale=0.125)
                                vs = v_sb[:, (t * HPC + h) * 65:(t * HPC + h) * 65 + 65]
                                if "ut" in phases:
                                    for half in range(2):
                                        nc.tensor.matmul(puts[half][:], vs,
                                                         e[:, 512 * half:512 * (half + 1)],
                                                         start=(t == 0), stop=(t == ST - 1),
                                                         skip_group_check=True)
                            for half in range(2 if "ut" in phases else 0):
                                qb = 2 * qbp + half
                                put = puts[half]
                                uraw = misc.tile([65, 512], F32, tag="uraw", name="uraw")
                                nc.vector.tensor_copy(uraw[:], put[:])
                                rec = misc.tile([1, 512], F32, tag="rec", name="rec")
                                nc.vector.reciprocal(rec[:], uraw[64:65, :])
                                bc = misc.tile([64, 512], F32, tag="bc", name="bc")
                                nc.gpsimd.partition_broadcast(bc[:], rec[:])
                                dst = ut_sb[ct][po:po + 64, 512 * qb:512 * (qb + 1)]
                                nc.vector.tensor_mul(dst, uraw[0:64, :], bc[:])
                                nc.vector.tensor_scalar_add(dst, dst, bv_sb[h][:])

                        if "out" in phases:
                            for st in range(8 * qbp, 8 * (qbp + 1)):
                                pos = [ps_gen.tile([128, 512], F32, tag="proj", name="po_")
                                       for _ in range(2)]
                                for cc in range(2):
                                    lhsT = ut_sb[cc][:, 128 * st:128 * (st + 1)]
                                    for nb in range(2):
                                        nc.tensor.matmul(
                                            pos[nb][:], lhsT,
                                            wo_sb[:, D * cc + 512 * nb: D * cc + 512 * (nb + 1)],
                                            start=(cc == 0), stop=(cc == 1))
                                for nb in range(2):
                                    stg = outst.tile([128, 512], MM, tag="stg", name="stg")
                                    nc.vector.tensor_copy(stg[:], pos[nb][:])
                                    nc.sync.dma_start(
                                        out[128 * st:128 * (st + 1), 512 * nb:512 * (nb + 1)],
                                        stg[:])

                    if "attn" in phases:
                        _attention(0)
                        _attention(1)
    nc.compile()
    return nc


def _v_scatter_ap(v_sb, t):
    """AP writing a [128, 256] chan-major block into the 65-strided V layout."""
    ap = v_sb[:, t * HPC * 65: t * HPC * 65 + HPC * 65]
    return ap.rearrange("p (h j) -> p h j", h=HPC)[:, :, 0:64]


def _prepare(q, k, v, Wq_w, Wq_b, Wk_w, Wk_b, Wv_w, Wv_b, Wo_w, Wo_b):
    f16 = np.float16
    pos = np.arange(1, S + 1, dtype=np.float32)
    theta = (BASE ** (-2.0 * np.arange(D // 2, dtype=np.float32) / D)).astype(np.float32)
    ang = theta[:, None] * pos[None, :]
    cosc = np.cos(ang).astype(f16)
    sinc = np.sin(ang).astype(f16)
    identity = np.eye(128, dtype=f16)

    per_batch = []
    for b in range(B):
        per_batch.append((
            np.ascontiguousarray(q[b].T[_PERM]).astype(f16),
            np.ascontiguousarray(k[b].T[_PERM]).astype(f16),
            np.ascontiguousarray(v[b].T).astype(f16),
        ))
    in_maps = []
    for c in range(N_CORES):
        b, g = divmod(c, GROUPS)
        rows = slice(CH * g, CH * (g + 1))
        qTb, kTb, vTb = per_batch[b]
        in_maps.append({
            "qT": qTb, "kT": kTb, "vT": vTb, "cosc": cosc, "sinc": sinc,
            "ident": identity,
            "wq": np.ascontiguousarray(Wq_w[rows, :].T[_PERM]).astype(f16),
            "wk": np.ascontiguousarray(Wk_w[rows, :].T[_PERM]).astype(f16),
            "wv": np.ascontiguousarray(Wv_w[rows, :].T).astype(f16),
            "wo": np.ascontiguousarray(Wo_w[:, rows].T).astype(f16),
            "bq": Wq_b[rows].astype(np.float32).reshape(2, 128, 1),
            "bk": Wk_b[rows].astype(np.float32).reshape(2, 128, 1),
            "bv": Wv_b[rows].astype(np.float32).reshape(HPC, 64, 1),
        })
    return in_maps


def kernel(q, k, v, Wq_w, Wq_b, Wk_w, Wk_b, Wv_w, Wv_b, Wo_w, Wo_b):
    global _PROG
    args = [np.asarray(x, dtype=np.float32) for x in
            (q, k, v, Wq_w, Wq_b, Wk_w, Wk_b, Wv_w, Wv_b, Wo_w, Wo_b)]
    if _PROG is None:
        _PROG = _build()
    in_maps = _prepare(*args)
    res = run_bass_kernel_spmd(_PROG, in_maps, core_ids=list(range(N_CORES)))
    kernel.last_results = res
    Wo_b32 = args[10]
    out = np.empty((B, S, D), dtype=np.float32)
    for b in range(B):
        acc = res.results[GROUPS * b]["out"].astype(np.float32)
        for g in range(1, GROUPS):
            acc += res.results[GROUPS * b + g]["out"]
        out[b] = acc + Wo_b32
    return out



# revision 33
# speedup vs baseline: 15197.5164x; 15197.5164x over previous
"""Multi-head attention (RoPE-full-dmodel variant) on 8 TRN2 NeuronCores.

Sharding: core c = (batch c//4, head-group c%4 of 4 heads).
 - W_q/W_k/W_v split column-wise by head (each core projects its 256 channels)
 - W_o split row-wise; per-core partial outputs summed on host (all-reduce at gather)

v1 restructure vs baseline (394us):
 - V projected directly seq-major (vT-tile stationary, wv moving): kills the
   32 PE transposes + chan-major staging copies; vT streamed per seq-tile
 - RoPE split into s-halves so K/Q projections (and the exp chain) start
   ~2x earlier; DVE/GpSimd column split rebalanced 896/128 (measured rates)
 - DMA emission ordered by first use: tables+kT in rope-pair order, then
   weights, qT, vT-per-tile, wo
 - softmax denominators: reciprocal_approx_fast straight from PSUM (replaces
   52us of [1,512] iterative reciprocals + staging copies)
 - kt/qt/ut/v split into per-block tiles so dependency tracking can't
   serialize attention on whole-tensor writes
"""
import os
import sys

for _p in ("/opt/trn_rl_repo", "/root/.axon_site/_ro/trn_rl_repo"):
    if os.path.isdir(_p) and _p not in sys.path:
        sys.path.insert(0, _p)

import numpy as np

import concourse.bacc as bacc
import concourse.tile as tile
import concourse.mybir as mybir
from concourse.bass_utils import run_bass_kernel_spmd

B, S, D = 2, 2048, 1024
H_TOT, DK = 16, 64
N_CORES, GROUPS = 8, 4
CH = 256            # channels (heads*dk) per core
KC = D // 128       # 8 d-model chunks
QB = S // 512       # 4 query blocks
ST = S // 128       # 16 seq tiles
HPC = 4             # heads per core
BASE = 10000.0

MM = mybir.dt.float16
F32 = mybir.dt.float32
AF = mybir.ActivationFunctionType

DVE_COLS = 896      # of each 1024-col half; rest on GpSimd (174 vs 30 Gel/s)

# even/odd permutation of the d_model axis: row r <- old d = 2r (r<512), 2(r-512)+1
_PERM = np.concatenate([np.arange(0, D, 2), np.arange(1, D, 2)])

_PROG = None


def _build():
    nc = bacc.Bacc("TRN2", target_bir_lowering=False, debug=False)
    qT = nc.dram_tensor("qT", (D, S), MM, kind="ExternalInput").ap()
    kT = nc.dram_tensor("kT", (D, S), MM, kind="ExternalInput").ap()
    vT = nc.dram_tensor("vT", (D, S), MM, kind="ExternalInput").ap()
    cosc = nc.dram_tensor("cosc", (D // 2, S), MM, kind="ExternalInput").ap()
    sinc = nc.dram_tensor("sinc", (D // 2, S), MM, kind="ExternalInput").ap()
    wq = nc.dram_tensor("wq", (D, CH), MM, kind="ExternalInput").ap()
    wk = nc.dram_tensor("wk", (D, CH), MM, kind="ExternalInput").ap()
    wv = nc.dram_tensor("wv", (D, CH), MM, kind="ExternalInput").ap()
    wo = nc.dram_tensor("wo", (CH, D), MM, kind="ExternalInput").ap()
    bq = nc.dram_tensor("bq", (2, 128, 1), F32, kind="ExternalInput").ap()
    bk = nc.dram_tensor("bk", (2, 128, 1), F32, kind="ExternalInput").ap()
    bv = nc.dram_tensor("bv", (HPC, 64, 1), F32, kind="ExternalInput").ap()
    out = nc.dram_tensor("out", (S, D), MM, kind="ExternalOutput").ap()
    dbg = None
    if os.environ.get("KERNEL_DEBUG"):
        dbg = {n: nc.dram_tensor(f"dbg_{n}", shp, MM, kind="ExternalOutput").ap()
               for n, shp in [("kt", (128, 512)), ("qt", (128, 1024)),
                              ("v", (128, 260)), ("ut", (128, 512))]}
        dbg["uraw"] = nc.dram_tensor("dbg_uraw", (65, 512), F32,
                                     kind="ExternalOutput").ap()
        dbg["bc"] = nc.dram_tensor("dbg_bc", (64, 512), F32,
                                   kind="ExternalOutput").ap()

    with tile.TileContext(nc) as tc:
      with (
          tc.tile_pool(name="consts", bufs=1) as consts,
          tc.tile_pool(name="qkv", bufs=1) as qkv,
          tc.tile_pool(name="expp", bufs=7) as expp,
          tc.tile_pool(name="vst", bufs=4) as vst,
          tc.tile_pool(name="misc", bufs=2) as misc,
          tc.tile_pool(name="outst", bufs=2) as outst,
          tc.tile_pool(name="ps_mix", bufs=4, space="PSUM") as ps_mix,
          tc.tile_pool(name="ps_sc", bufs=2, space="PSUM") as ps_sc,
      ):
        # persistent activation tiles, split per block for fine-grained deps
        v_t = [qkv.tile([128, HPC * 65], MM, tag=f"v{t}", name=f"v{t}")
               for t in range(ST)]
        for t in range(ST):
            nc.vector.memset(
                v_t[t][:].rearrange("p (h j) -> p h j", j=65)[:, :, 64], 1.0)
        kt_t = [[qkv.tile([128, 512], MM, tag=f"kt{c}_{b}", name=f"kt{c}_{b}")
                 for b in range(QB)] for c in range(2)]
        qt_t = [[qkv.tile([128, 1024], MM, tag=f"qt{c}_{p}", name=f"qt{c}_{p}")
                 for p in range(2)] for c in range(2)]
        ut_t = [[qkv.tile([128, 512], MM, tag=f"ut{c}_{b}", name=f"ut{c}_{b}")
                 for b in range(QB)] for c in range(2)]

        wq_sb = consts.tile([128, KC * CH], MM, tag="wq")
        wk_sb = consts.tile([128, KC * CH], MM, tag="wk")
        wv_sb = consts.tile([128, KC * CH], MM, tag="wv")
        wo_sb = consts.tile([128, 2 * D], MM, tag="wo")
        bq_sb, bk_sb, bv_sb = [], [], []

        with tc.tile_pool(name="p1", bufs=1) as p1:
            # ---------- DMA emission: order = service order ----------
            # chunk-group transfers: one 1MB gathered DMA per (tensor, half,
            # 4-chunk group), ordered to match the rope schedule k0,k1,q0,q1
            cos_g, sin_g = [None, None], [None, None]
            xk_g = [[None, None] for _ in range(2)]   # [half][group]
            xq_g = [[None, None] for _ in range(2)]

            def _gld(name, src, rows, half, tag):
                lo = 1024 * half
                x_ = p1.tile([128, 4096], MM, tag=tag, name=tag)
                s = src[rows:rows + 512, lo:lo + 1024].rearrange(
                    "(c p) n -> p c n", p=128)
                nc.sync.dma_start(x_[:].rearrange("p (c n) -> p c n", n=1024), s)
                return x_

            cos_g[0] = _gld("c", cosc, 0, 0, "cos_0")
            sin_g[0] = _gld("s", sinc, 0, 0, "sin_0")
            xk_g[0][0] = _gld("k", kT, 0, 0, "xk_0_0")
            xk_g[0][1] = _gld("k", kT, 512, 0, "xk_0_1")
            for c in range(KC):
                nc.sync.dma_start(wk_sb[:, CH * c:CH * (c + 1)],
                                  wk[128 * c:128 * (c + 1), :])
            cos_g[1] = _gld("c", cosc, 0, 1, "cos_1")
            sin_g[1] = _gld("s", sinc, 0, 1, "sin_1")
            xk_g[1][0] = _gld("k", kT, 0, 1, "xk_1_0")
            xk_g[1][1] = _gld("k", kT, 512, 1, "xk_1_1")
            xq_g[0][0] = _gld("q", qT, 0, 0, "xq_0_0")
            xq_g[0][1] = _gld("q", qT, 512, 0, "xq_0_1")
            for c in range(KC):
                nc.sync.dma_start(wq_sb[:, CH * c:CH * (c + 1)],
                                  wq[128 * c:128 * (c + 1), :])
            for c in range(2):
                t_ = consts.tile([128, 1], F32, tag=f"bq{c}", name=f"bq{c}")
                nc.sync.dma_start(t_[:], bq[c])
                bq_sb.append(t_)
                t_ = consts.tile([128, 1], F32, tag=f"bk{c}", name=f"bk{c}")
                nc.sync.dma_start(t_[:], bk[c])
                bk_sb.append(t_)
            for h in range(HPC):
                t_ = consts.tile([64, 1], F32, tag=f"bv{h}", name=f"bv{h}")
                nc.sync.dma_start(t_[:], bv[h])
                bv_sb.append(t_)
            for c in range(KC):
                nc.sync.dma_start(wv_sb[:, CH * c:CH * (c + 1)],
                                  wv[128 * c:128 * (c + 1), :])
            # vT per seq-tile-pair: gathered DMA, consumed streaming by the
            # inline vproj (tiles in outer pool: p1 closes before attention)
            xvt2 = []
            for tp in range(ST // 2):
                x_ = vst.tile([128, 2048], MM, tag="xvt", name=f"xvt{tp}")
                src = vT[:, 256 * tp:256 * (tp + 1)].rearrange(
                    "(c p) n -> p c n", p=128)
                nc.sync.dma_start(x_[:].rearrange("p (c n) -> p c n", n=256), src)
                xvt2.append(x_)
            xq_g[1][0] = _gld("q", qT, 0, 1, "xq_1_0")
            xq_g[1][1] = _gld("q", qT, 512, 1, "xq_1_1")
            for c in range(2):
                nc.sync.dma_start(wo_sb[:, D * c:D * (c + 1)],
                                  wo[128 * c:128 * (c + 1), :])

            # ---------- rope (per s-half, DVE only) ----------
            def _rope_half(xg, half):
                cs = lambda t: cos_g[half][:, 1024 * t:1024 * (t + 1)]
                ss = lambda t: sin_g[half][:, 1024 * t:1024 * (t + 1)]
                xx = lambda a: xg[half][a // 4][:, 1024 * (a % 4):1024 * (a % 4 + 1)]
                roped = [None] * KC
                for a in (0, 1, 4, 5):
                    b_ = a + 2
                    ca, cb = a % 4, b_ % 4
                    xa, xb = xx(a), xx(b_)
                    t1 = p1.tile([128, 1024], MM, tag="tmp", name="t1", bufs=3)
                    nc.vector.tensor_mul(t1[:], xa, cs(ca))
                    t2 = p1.tile([128, 1024], MM, tag="tmp", name="t2", bufs=3)
                    nc.vector.tensor_mul(t2[:], xb, ss(ca))
                    ra = p1.tile([128, 1024], MM, tag="roped", name="ra", bufs=6)
                    nc.vector.tensor_sub(ra[:], t1[:], t2[:])
                    t3 = p1.tile([128, 1024], MM, tag="tmp", name="t3", bufs=3)
                    nc.vector.tensor_mul(t3[:], xb, cs(cb))
                    t4 = p1.tile([128, 1024], MM, tag="tmp", name="t4", bufs=3)
                    nc.vector.tensor_mul(t4[:], xa, ss(cb))
                    rb = p1.tile([128, 1024], MM, tag="roped", name="rb", bufs=6)
                    nc.vector.tensor_add(rb[:], t3[:], t4[:])
                    roped[a], roped[b_] = ra, rb
                return roped

            def _proj(roped, w_sb, b_sb, dst_fn, half):
                for c in range(2):
                    psums = [ps_mix.tile([128, 512], F32, tag="mix", name="psum")
                             for _ in range(2)]
                    for d in range(KC):
                        lhsT = w_sb[:, CH * d + 128 * c: CH * d + 128 * (c + 1)]
                        for i in range(2):
                            nc.tensor.matmul(psums[i][:], lhsT,
                                             roped[d][:, 512 * i:512 * (i + 1)],
                                             start=(d == 0), stop=(d == KC - 1))
                    for i in range(2):
                        nc.scalar.activation(dst_fn(c, 2 * half + i), psums[i][:],
                                             AF.Identity, bias=b_sb[c][:])

            def _kdst(c, s):
                return kt_t[c][s][:]

            def _qdst(c, s):
                return qt_t[c][s // 2][:, 512 * (s % 2):512 * (s % 2 + 1)]

            rk0 = _rope_half(xk_g, 0)
            _proj(rk0, wk_sb, bk_sb, _kdst, 0)
            rk1 = _rope_half(xk_g, 1)
            _proj(rk1, wk_sb, bk_sb, _kdst, 1)
            rq0 = _rope_half(xq_g, 0)
            _proj(rq0, wq_sb, bq_sb, _qdst, 0)
            # q-h1 feeds only qbp1 (needed ~70us later): demote its priority so
            # its projections fill PE slack instead of stalling the exp chain
            with tc.high_priority(offset=-50000):
                rq1 = _rope_half(xq_g, 1)
                _proj(rq1, wq_sb, bq_sb, _qdst, 1)

        # ---------- V projection, directly seq-major (emitted inline) ----------
        def _vproj_t(t):
            pv = ps_mix.tile([128, 256], F32, tag="mix", name="pv")
            xt = xvt2[t // 2]
            for d in range(KC):
                lhsT = xt[:, 256 * d + 128 * (t % 2):256 * d + 128 * (t % 2) + 128]
                nc.tensor.matmul(pv[:], lhsT,
                                 wv_sb[:, CH * d:CH * (d + 1)],
                                 start=(d == 0), stop=(d == KC - 1))
            dst = v_t[t][:].rearrange("p (h j) -> p h j", j=65)[:, :, 0:64]
            nc.scalar.copy(dst, pv[:])

        # ---------- attention + output projection ----------
        # Heads are processed in pairs (A=2hp on partitions 0-63, B=2hp+1 on
        # 64-127): the two score matmuls go to disjoint PE row-groups and run
        # concurrently; one exp covers both heads at FD=1024. Query halves are
        # sequential so only 2 UT accumulators are live (leaves ps_mix slots
        # for vproj/outproj).
        ones64 = consts.tile([1, 64], MM, tag="ones64")
        nc.vector.memset(ones64[:], 1.0)
        dbg_nrm = {}

        def _normalize(h, qb, put):
            # broadcast den across 64 partitions with a K=1 PE matmul
            # (gpsimd partition_broadcast thrashes Q7 IRAM library reloads)
            ct, po = h // 2, 64 * (h % 2)
            uraw = misc.tile([65, 512], F32, tag="uraw", name="uraw")
            nc.vector.tensor_copy(uraw[:], put[:])
            den16 = misc.tile([1, 512], MM, tag="den16", name="den16")
            nc.vector.tensor_copy(den16[:], uraw[64:65, :])
            bcd = ps_mix.tile([64, 512], F32, tag="mix", name="bcd")
            nc.tensor.matmul(bcd[:], ones64[:], den16[:], start=True, stop=True)
            bc = misc.tile([64, 512], F32, tag="bc", name="bc")
            nc.vector.reciprocal_approx_fast(bc[:], bcd[:])
            dst = ut_t[ct][qb][po:po + 64, :]
            nc.vector.tensor_mul(dst, uraw[0:64, :], bc[:])
            nc.vector.tensor_scalar_add(dst, dst, bv_sb[h][:])
            if dbg is not None and h == 0 and qb == 0:
                dbg_nrm["uraw"] = uraw
                dbg_nrm["bc"] = bc

        outproj_pending = []

        def _flush_outproj():
            while outproj_pending:
                _outproj_qb(outproj_pending.pop(0))

        def _attention(qbp):
            for half in range(2):
                qb = 2 * qbp + half
                for hp in range(2):
                    hA, hB, ct = 2 * hp, 2 * hp + 1, hp
                    qA = qt_t[ct][qbp][0:64, 512 * half:512 * (half + 1)]
                    qB = qt_t[ct][qbp][64:128, 512 * half:512 * (half + 1)]
                    putA = ps_mix.tile([65, 512], F32, tag="mix", name="putA")
                    putB = ps_mix.tile([65, 512], F32, tag="mix", name="putB")
                    for t in range(ST):
                        ktile = kt_t[ct][t // 4]
                        co = 128 * (t % 4)
                        psc = ps_sc.tile([128, 1024], F32, tag="sc", name="psc")
                        nc.tensor.matmul(psc[:, 0:512],
                                         ktile[0:64, co:co + 128], qA,
                                         start=True, stop=True)
                        nc.tensor.matmul(psc[:, 512:1024],
                                         ktile[64:128, co:co + 128], qB,
                                         start=True, stop=True)
                        e = expp.tile([128, 1024], MM, tag="e", name="e")
                        nc.scalar.activation(e[:], psc[:], AF.Exp, scale=0.125)
                        if qbp == 0 and half == 0 and hp == 0:
                            _vproj_t(t)
                        nc.tensor.matmul(putA[:], v_t[t][:, 65 * hA:65 * hA + 65],
                                         e[:, 0:512],
                                         start=(t == 0), stop=(t == ST - 1),
                                         skip_group_check=True)
                        nc.tensor.matmul(putB[:], v_t[t][:, 65 * hB:65 * hB + 65],
                                         e[:, 512:1024],
                                         start=(t == 0), stop=(t == ST - 1),
                                         skip_group_check=True)
                    _normalize(hA, qb, putA)
                    _normalize(hB, qb, putB)
                    if hp == 0:
                        # previous qb's output projection: emitted mid-half so
                        # its PE priority sits behind this half's first scores
                        _flush_outproj()
                outproj_pending.append(qb)

        def _outproj_qb(qb):
            for si in range(4):
                st = 4 * qb + si
                pos = [ps_mix.tile([128, 512], F32, tag="mix", name="po_")
                       for _ in range(2)]
                for cc in range(2):
                    lhsT = ut_t[cc][qb][:, 128 * si:128 * (si + 1)]
                    for nb in range(2):
                        nc.tensor.matmul(
                            pos[nb][:], lhsT,
                            wo_sb[:, D * cc + 512 * nb: D * cc + 512 * (nb + 1)],
                            start=(cc == 0), stop=(cc == 1))
                for nb in range(2):
                    stg = outst.tile([128, 512], MM, tag="stg", name="stg")
                    nc.vector.tensor_copy(stg[:], pos[nb][:])
                    nc.sync.dma_start(
                        out[128 * st:128 * (st + 1), 512 * nb:512 * (nb + 1)],
                        stg[:])

        _attention(0)
        _attention(1)
        _flush_outproj()

        if dbg is not None:
            nc.sync.dma_start(dbg["kt"], kt_t[0][0][:])
            nc.sync.dma_start(dbg["qt"], qt_t[0][0][:])
            nc.sync.dma_start(dbg["v"], v_t[0][:])
            nc.sync.dma_start(dbg["ut"], ut_t[0][0][:])
            nc.sync.dma_start(dbg["uraw"], dbg_nrm["uraw"][:])
            nc.sync.dma_start(dbg["bc"], dbg_nrm["bc"][:])

    nc.compile()
    return nc


def _prepare(q, k, v, Wq_w, Wq_b, Wk_w, Wk_b, Wv_w, Wv_b, Wo_w, Wo_b):
    f16 = np.float16
    pos = np.arange(1, S + 1, dtype=np.float32)
    theta = (BASE ** (-2.0 * np.arange(D // 2, dtype=np.float32) / D)).astype(np.float32)
    ang = theta[:, None] * pos[None, :]
    cosc = np.cos(ang).astype(f16)
    sinc = np.sin(ang).astype(f16)

    per_batch = []
    for b in range(B):
        per_batch.append((
            np.ascontiguousarray(q[b].T[_PERM]).astype(f16),
            np.ascontiguousarray(k[b].T[_PERM]).astype(f16),
            np.ascontiguousarray(v[b].T).astype(f16),
        ))
    in_maps = []
    for c in range(N_CORES):
        b, g = divmod(c, GROUPS)
        rows = slice(CH * g, CH * (g + 1))
        qTb, kTb, vTb = per_batch[b]
        in_maps.append({
            "qT": qTb, "kT": kTb, "vT": vTb, "cosc": cosc, "sinc": sinc,
            "wq": np.ascontiguousarray(Wq_w[rows, :].T[_PERM]).astype(f16),
            "wk": np.ascontiguousarray(Wk_w[rows, :].T[_PERM]).astype(f16),
            "wv": np.ascontiguousarray(Wv_w[rows, :].T).astype(f16),
            "wo": np.ascontiguousarray(Wo_w[:, rows].T).astype(f16),
            "bq": Wq_b[rows].astype(np.float32).reshape(2, 128, 1),
            "bk": Wk_b[rows].astype(np.float32).reshape(2, 128, 1),
            "bv": Wv_b[rows].astype(np.float32).reshape(HPC, 64, 1),
        })
    return in_maps


def kernel(q, k, v, Wq_w, Wq_b, Wk_w, Wk_b, Wv_w, Wv_b, Wo_w, Wo_b):
    global _PROG
    args = [np.asarray(x, dtype=np.float32) for x in
            (q, k, v, Wq_w, Wq_b, Wk_w, Wk_b, Wv_w, Wv_b, Wo_w, Wo_b)]
    if _PROG is None:
        _PROG = _build()
    in_maps = _prepare(*args)
    res = run_bass_kernel_spmd(_PROG, in_maps, core_ids=list(range(N_CORES)))
    kernel.last_results = res
    Wo_b32 = args[10]
    out = np.empty((B, S, D), dtype=np.float32)
    for b in range(B):
        acc = res.results[GROUPS * b]["out"].astype(np.float32)
        for g in range(1, GROUPS):
            acc += res.results[GROUPS * b + g]["out"]
        out[b] = acc + Wo_b32
    return out


# revision 34
# speedup vs baseline: 15218.6711x; 1.0014x over previous
"""Multi-head attention (RoPE-full-dmodel variant) on 8 TRN2 NeuronCores.

Sharding: core c = (batch c//4, head-group c%4 of 4 heads).
 - W_q/W_k/W_v split column-wise by head (each core projects its 256 channels)
 - W_o split row-wise; per-core partial outputs summed on host (all-reduce at gather)

v1 restructure vs baseline (394us):
 - V projected directly seq-major (vT-tile stationary, wv moving): kills the
   32 PE transposes + chan-major staging copies; vT streamed per seq-tile
 - RoPE split into s-halves so K/Q projections (and the exp chain) start
   ~2x earlier; DVE/GpSimd column split rebalanced 896/128 (measured rates)
 - DMA emission ordered by first use: tables+kT in rope-pair order, then
   weights, qT, vT-per-tile, wo
 - softmax denominators: reciprocal_approx_fast straight from PSUM (replaces
   52us of [1,512] iterative reciprocals + staging copies)
 - kt/qt/ut/v split into per-block tiles so dependency tracking can't
   serialize attention on whole-tensor writes
"""
import os
import sys

for _p in ("/opt/trn_rl_repo", "/root/.axon_site/_ro/trn_rl_repo"):
    if os.path.isdir(_p) and _p not in sys.path:
        sys.path.insert(0, _p)

import numpy as np

import concourse.bacc as bacc
import concourse.tile as tile
import concourse.mybir as mybir
from concourse.bass_utils import run_bass_kernel_spmd

B, S, D = 2, 2048, 1024
H_TOT, DK = 16, 64
N_CORES, GROUPS = 8, 4
CH = 256            # channels (heads*dk) per core
KC = D // 128       # 8 d-model chunks
QB = S // 512       # 4 query blocks
ST = S // 128       # 16 seq tiles
HPC = 4             # heads per core
BASE = 10000.0

MM = mybir.dt.float16
F32 = mybir.dt.float32
AF = mybir.ActivationFunctionType

DVE_COLS = 896      # of each 1024-col half; rest on GpSimd (174 vs 30 Gel/s)

# even/odd permutation of the d_model axis: row r <- old d = 2r (r<512), 2(r-512)+1
_PERM = np.concatenate([np.arange(0, D, 2), np.arange(1, D, 2)])

_PROG = None


def _build():
    nc = bacc.Bacc("TRN2", target_bir_lowering=False, debug=False)
    qT = nc.dram_tensor("qT", (D, S), MM, kind="ExternalInput").ap()
    kT = nc.dram_tensor("kT", (D, S), MM, kind="ExternalInput").ap()
    vT = nc.dram_tensor("vT", (D, S), MM, kind="ExternalInput").ap()
    cosc = nc.dram_tensor("cosc", (D // 2, S), MM, kind="ExternalInput").ap()
    sinc = nc.dram_tensor("sinc", (D // 2, S), MM, kind="ExternalInput").ap()
    wq = nc.dram_tensor("wq", (D, CH), MM, kind="ExternalInput").ap()
    wk = nc.dram_tensor("wk", (D, CH), MM, kind="ExternalInput").ap()
    wv = nc.dram_tensor("wv", (D, CH), MM, kind="ExternalInput").ap()
    wo = nc.dram_tensor("wo", (CH, D), MM, kind="ExternalInput").ap()
    bq = nc.dram_tensor("bq", (2, 128, 1), F32, kind="ExternalInput").ap()
    bk = nc.dram_tensor("bk", (2, 128, 1), F32, kind="ExternalInput").ap()
    bv = nc.dram_tensor("bv", (HPC, 64, 1), F32, kind="ExternalInput").ap()
    out = nc.dram_tensor("out", (S, D), MM, kind="ExternalOutput").ap()
    dbg = None
    if os.environ.get("KERNEL_DEBUG"):
        dbg = {n: nc.dram_tensor(f"dbg_{n}", shp, MM, kind="ExternalOutput").ap()
               for n, shp in [("kt", (128, 512)), ("qt", (128, 1024)),
                              ("v", (128, 260)), ("ut", (128, 512))]}
        dbg["uraw"] = nc.dram_tensor("dbg_uraw", (65, 512), F32,
                                     kind="ExternalOutput").ap()
        dbg["bc"] = nc.dram_tensor("dbg_bc", (64, 512), F32,
                                   kind="ExternalOutput").ap()

    with tile.TileContext(nc) as tc:
      with (
          tc.tile_pool(name="consts", bufs=1) as consts,
          tc.tile_pool(name="qkv", bufs=1) as qkv,
          tc.tile_pool(name="expp", bufs=7) as expp,
          tc.tile_pool(name="vst", bufs=4) as vst,
          tc.tile_pool(name="misc", bufs=2) as misc,
          tc.tile_pool(name="outst", bufs=2) as outst,
          tc.tile_pool(name="ps_mix", bufs=4, space="PSUM") as ps_mix,
          tc.tile_pool(name="ps_sc", bufs=2, space="PSUM") as ps_sc,
      ):
        # persistent activation tiles, split per block for fine-grained deps
        v_t = [qkv.tile([128, HPC * 65], MM, tag=f"v{t}", name=f"v{t}")
               for t in range(ST)]
        for t in range(ST):
            nc.vector.memset(
                v_t[t][:].rearrange("p (h j) -> p h j", j=65)[:, :, 64], 1.0)
        kt_t = [[qkv.tile([128, 512], MM, tag=f"kt{c}_{b}", name=f"kt{c}_{b}")
                 for b in range(QB)] for c in range(2)]
        qt_t = [[qkv.tile([128, 1024], MM, tag=f"qt{c}_{p}", name=f"qt{c}_{p}")
                 for p in range(2)] for c in range(2)]
        ut_t = [[qkv.tile([128, 512], MM, tag=f"ut{c}_{b}", name=f"ut{c}_{b}")
                 for b in range(QB)] for c in range(2)]

        wq_sb = consts.tile([128, KC * CH], MM, tag="wq")
        wk_sb = consts.tile([128, KC * CH], MM, tag="wk")
        wv_sb = consts.tile([128, KC * CH], MM, tag="wv")
        wo_sb = consts.tile([128, 2 * D], MM, tag="wo")
        bq_sb, bk_sb, bv_sb = [], [], []

        with tc.tile_pool(name="p1", bufs=1) as p1:
            # ---------- DMA emission: order = service order ----------
            # chunk-group transfers: one 1MB gathered DMA per (tensor, half,
            # 4-chunk group), ordered to match the rope schedule k0,k1,q0,q1
            cos_g, sin_g = [None, None], [None, None]
            xk_g = [[None, None] for _ in range(2)]   # [half][group]
            xq_g = [[None, None] for _ in range(2)]

            def _gld(name, src, rows, half, tag):
                lo = 1024 * half
                x_ = p1.tile([128, 4096], MM, tag=tag, name=tag)
                s = src[rows:rows + 512, lo:lo + 1024].rearrange(
                    "(c p) n -> p c n", p=128)
                nc.sync.dma_start(x_[:].rearrange("p (c n) -> p c n", n=1024), s)
                return x_

            cos_g[0] = _gld("c", cosc, 0, 0, "cos_0")
            sin_g[0] = _gld("s", sinc, 0, 0, "sin_0")
            xk_g[0][0] = _gld("k", kT, 0, 0, "xk_0_0")
            xk_g[0][1] = _gld("k", kT, 512, 0, "xk_0_1")
            for c in range(KC):
                nc.sync.dma_start(wk_sb[:, CH * c:CH * (c + 1)],
                                  wk[128 * c:128 * (c + 1), :])
            cos_g[1] = _gld("c", cosc, 0, 1, "cos_1")
            sin_g[1] = _gld("s", sinc, 0, 1, "sin_1")
            xk_g[1][0] = _gld("k", kT, 0, 1, "xk_1_0")
            xk_g[1][1] = _gld("k", kT, 512, 1, "xk_1_1")
            xq_g[0][0] = _gld("q", qT, 0, 0, "xq_0_0")
            xq_g[0][1] = _gld("q", qT, 512, 0, "xq_0_1")
            for c in range(KC):
                nc.sync.dma_start(wq_sb[:, CH * c:CH * (c + 1)],
                                  wq[128 * c:128 * (c + 1), :])
            for c in range(2):
                t_ = consts.tile([128, 1], F32, tag=f"bq{c}", name=f"bq{c}")
                nc.sync.dma_start(t_[:], bq[c])
                bq_sb.append(t_)
                t_ = consts.tile([128, 1], F32, tag=f"bk{c}", name=f"bk{c}")
                nc.sync.dma_start(t_[:], bk[c])
                bk_sb.append(t_)
            for h in range(HPC):
                t_ = consts.tile([64, 1], F32, tag=f"bv{h}", name=f"bv{h}")
                nc.sync.dma_start(t_[:], bv[h])
                bv_sb.append(t_)
            for c in range(KC):
                nc.sync.dma_start(wv_sb[:, CH * c:CH * (c + 1)],
                                  wv[128 * c:128 * (c + 1), :])
            # vT per seq-tile-pair: gathered DMA, consumed streaming by the
            # inline vproj (tiles in outer pool: p1 closes before attention)
            xvt2 = []
            for tp in range(ST // 2):
                x_ = vst.tile([128, 2048], MM, tag="xvt", name=f"xvt{tp}")
                src = vT[:, 256 * tp:256 * (tp + 1)].rearrange(
                    "(c p) n -> p c n", p=128)
                nc.sync.dma_start(x_[:].rearrange("p (c n) -> p c n", n=256), src)
                xvt2.append(x_)
            xq_g[1][0] = _gld("q", qT, 0, 1, "xq_1_0")
            xq_g[1][1] = _gld("q", qT, 512, 1, "xq_1_1")
            for c in range(2):
                nc.sync.dma_start(wo_sb[:, D * c:D * (c + 1)],
                                  wo[128 * c:128 * (c + 1), :])

            # ---------- rope (per s-half, DVE only) ----------
            def _rope_half(xg, half):
                cs = lambda t: cos_g[half][:, 1024 * t:1024 * (t + 1)]
                ss = lambda t: sin_g[half][:, 1024 * t:1024 * (t + 1)]
                xx = lambda a: xg[half][a // 4][:, 1024 * (a % 4):1024 * (a % 4 + 1)]
                roped = [None] * KC
                for a in (0, 1, 4, 5):
                    b_ = a + 2
                    ca, cb = a % 4, b_ % 4
                    xa, xb = xx(a), xx(b_)
                    t1 = p1.tile([128, 1024], MM, tag="tmp", name="t1", bufs=3)
                    nc.vector.tensor_mul(t1[:], xa, cs(ca))
                    t2 = p1.tile([128, 1024], MM, tag="tmp", name="t2", bufs=3)
                    nc.vector.tensor_mul(t2[:], xb, ss(ca))
                    ra = p1.tile([128, 1024], MM, tag="roped", name="ra", bufs=6)
                    nc.vector.tensor_sub(ra[:], t1[:], t2[:])
                    t3 = p1.tile([128, 1024], MM, tag="tmp", name="t3", bufs=3)
                    nc.vector.tensor_mul(t3[:], xb, cs(cb))
                    t4 = p1.tile([128, 1024], MM, tag="tmp", name="t4", bufs=3)
                    nc.vector.tensor_mul(t4[:], xa, ss(cb))
                    rb = p1.tile([128, 1024], MM, tag="roped", name="rb", bufs=6)
                    nc.vector.tensor_add(rb[:], t3[:], t4[:])
                    roped[a], roped[b_] = ra, rb
                return roped

            def _proj(roped, w_sb, b_sb, dst_fn, half):
                for c in range(2):
                    for i in range(2):
                        psum = ps_mix.tile([128, 512], F32, tag="mix", name="psum")
                        for d in range(KC):
                            lhsT = w_sb[:, CH * d + 128 * c: CH * d + 128 * (c + 1)]
                            nc.tensor.matmul(psum[:], lhsT,
                                             roped[d][:, 512 * i:512 * (i + 1)],
                                             start=(d == 0), stop=(d == KC - 1))
                        nc.scalar.activation(dst_fn(c, 2 * half + i), psum[:],
                                             AF.Identity, bias=b_sb[c][:])

            def _kdst(c, s):
                return kt_t[c][s][:]

            def _qdst(c, s):
                return qt_t[c][s // 2][:, 512 * (s % 2):512 * (s % 2 + 1)]

            rk0 = _rope_half(xk_g, 0)
            _proj(rk0, wk_sb, bk_sb, _kdst, 0)
            rk1 = _rope_half(xk_g, 1)
            _proj(rk1, wk_sb, bk_sb, _kdst, 1)
            rq0 = _rope_half(xq_g, 0)
            _proj(rq0, wq_sb, bq_sb, _qdst, 0)
            # q-h1 feeds only qbp1 (needed ~70us later): demote its priority so
            # its projections fill PE slack instead of stalling the exp chain
            with tc.high_priority(offset=-50000):
                rq1 = _rope_half(xq_g, 1)
                _proj(rq1, wq_sb, bq_sb, _qdst, 1)

        # ---------- V projection, directly seq-major ----------
        def _vproj_t(t):
            pv = ps_mix.tile([128, 256], F32, tag="mix", name="pv")
            xt = xvt2[t // 2]
            for d in range(KC):
                lhsT = xt[:, 256 * d + 128 * (t % 2):256 * d + 128 * (t % 2) + 128]
                nc.tensor.matmul(pv[:], lhsT,
                                 wv_sb[:, CH * d:CH * (d + 1)],
                                 start=(d == 0), stop=(d == KC - 1))
            dst = v_t[t][:].rearrange("p (h j) -> p h j", j=65)[:, :, 0:64]
            nc.scalar.copy(dst, pv[:])

        # ---------- attention + output projection ----------
        # Heads are processed in pairs (A=2hp on partitions 0-63, B=2hp+1 on
        # 64-127): the two score matmuls go to disjoint PE row-groups and run
        # concurrently; one exp covers both heads at FD=1024. Query halves are
        # sequential so only 2 UT accumulators are live (leaves ps_mix slots
        # for vproj/outproj).
        for _vt in range(ST):
            _vproj_t(_vt)

        ones64 = consts.tile([1, 64], MM, tag="ones64")
        nc.vector.memset(ones64[:], 1.0)
        dbg_nrm = {}

        def _norm_copy(put):
            uraw = misc.tile([65, 512], F32, tag="uraw", name="uraw")
            nc.vector.tensor_copy(uraw[:], put[:])
            return uraw

        def _normalize(h, qb, uraw):
            # broadcast den across 64 partitions with a K=1 PE matmul
            # (gpsimd partition_broadcast thrashes Q7 IRAM library reloads)
            ct, po = h // 2, 64 * (h % 2)
            den16 = misc.tile([1, 512], MM, tag="den16", name="den16")
            nc.vector.tensor_copy(den16[:], uraw[64:65, :])
            bcd = ps_mix.tile([64, 512], F32, tag="mix", name="bcd")
            nc.tensor.matmul(bcd[:], ones64[:], den16[:], start=True, stop=True)
            bc = misc.tile([64, 512], F32, tag="bc", name="bc")
            nc.vector.reciprocal_approx_fast(bc[:], bcd[:])
            dst = ut_t[ct][qb][po:po + 64, :]
            nc.vector.tensor_mul(dst, uraw[0:64, :], bc[:])
            nc.vector.tensor_scalar_add(dst, dst, bv_sb[h][:])
            if dbg is not None and h == 0 and qb == 0:
                dbg_nrm["uraw"] = uraw
                dbg_nrm["bc"] = bc

        outproj_pending = []

        def _flush_outproj():
            while outproj_pending:
                _outproj_qb(outproj_pending.pop(0))

        def _attention(qbp):
            for half in range(2):
                qb = 2 * qbp + half
                for hp in range(2):
                    hA, hB, ct = 2 * hp, 2 * hp + 1, hp
                    qA = qt_t[ct][qbp][0:64, 512 * half:512 * (half + 1)]
                    qB = qt_t[ct][qbp][64:128, 512 * half:512 * (half + 1)]
                    putA = ps_mix.tile([65, 512], F32, tag="mix", name="putA")
                    putB = ps_mix.tile([65, 512], F32, tag="mix", name="putB")
                    for t in range(ST):
                        ktile = kt_t[ct][t // 4]
                        co = 128 * (t % 4)
                        psc = ps_sc.tile([128, 1024], F32, tag="sc", name="psc")
                        nc.tensor.matmul(psc[:, 0:512],
                                         ktile[0:64, co:co + 128], qA,
                                         start=True, stop=True)
                        nc.tensor.matmul(psc[:, 512:1024],
                                         ktile[64:128, co:co + 128], qB,
                                         start=True, stop=True)
                        e = expp.tile([128, 1024], MM, tag="e", name="e")
                        nc.scalar.activation(e[:], psc[:], AF.Exp, scale=0.125)
                        nc.tensor.matmul(putA[:], v_t[t][:, 65 * hA:65 * hA + 65],
                                         e[:, 0:512],
                                         start=(t == 0), stop=(t == ST - 1),
                                         skip_group_check=True)
                        nc.tensor.matmul(putB[:], v_t[t][:, 65 * hB:65 * hB + 65],
                                         e[:, 512:1024],
                                         start=(t == 0), stop=(t == ST - 1),
                                         skip_group_check=True)
                    urA = _norm_copy(putA)
                    urB = _norm_copy(putB)
                    _normalize(hA, qb, urA)
                    _normalize(hB, qb, urB)
                    if hp == 0:
                        # previous qb's output projection: emitted mid-half so
                        # its PE priority sits behind this half's first scores
                        _flush_outproj()
                outproj_pending.append(qb)

        def _outproj_qb(qb):
            for si in range(4):
                st = 4 * qb + si
                pos = [ps_mix.tile([128, 512], F32, tag="mix", name="po_")
                       for _ in range(2)]
                for cc in range(2):
                    lhsT = ut_t[cc][qb][:, 128 * si:128 * (si + 1)]
                    for nb in range(2):
                        nc.tensor.matmul(
                            pos[nb][:], lhsT,
                            wo_sb[:, D * cc + 512 * nb: D * cc + 512 * (nb + 1)],
                            start=(cc == 0), stop=(cc == 1))
                for nb in range(2):
                    stg = outst.tile([128, 512], MM, tag="stg", name="stg")
                    nc.vector.tensor_copy(stg[:], pos[nb][:])
                    nc.sync.dma_start(
                        out[128 * st:128 * (st + 1), 512 * nb:512 * (nb + 1)],
                        stg[:])

        _attention(0)
        _attention(1)
        _flush_outproj()

        if dbg is not None:
            nc.sync.dma_start(dbg["kt"], kt_t[0][0][:])
            nc.sync.dma_start(dbg["qt"], qt_t[0][0][:])
            nc.sync.dma_start(dbg["v"], v_t[0][:])
            nc.sync.dma_start(dbg["ut"], ut_t[0][0][:])
            nc.sync.dma_start(dbg["uraw"], dbg_nrm["uraw"][:])
            nc.sync.dma_start(dbg["bc"], dbg_nrm["bc"][:])

    nc.compile()
    return nc


def _prepare(q, k, v, Wq_w, Wq_b, Wk_w, Wk_b, Wv_w, Wv_b, Wo_w, Wo_b):
    f16 = np.float16
    pos = np.arange(1, S + 1, dtype=np.float32)
    theta = (BASE ** (-2.0 * np.arange(D // 2, dtype=np.float32) / D)).astype(np.float32)
    ang = theta[:, None] * pos[None, :]
    cosc = np.cos(ang).astype(f16)
    sinc = np.sin(ang).astype(f16)

    per_batch = []
    for b in range(B):
        per_batch.append((
            np.ascontiguousarray(q[b].T[_PERM]).astype(f16),
            np.ascontiguousarray(k[b].T[_PERM]).astype(f16),
            np.ascontiguousarray(v[b].T).astype(f16),
        ))
    in_maps = []
    for c in range(N_CORES):
        b, g = divmod(c, GROUPS)
        rows = slice(CH * g, CH * (g + 1))
        qTb, kTb, vTb = per_batch[b]
        in_maps.append({
            "qT": qTb, "kT": kTb, "vT": vTb, "cosc": cosc, "sinc": sinc,
            "wq": np.ascontiguousarray(Wq_w[rows, :].T[_PERM]).astype(f16),
            "wk": np.ascontiguousarray(Wk_w[rows, :].T[_PERM]).astype(f16),
            "wv": np.ascontiguousarray(Wv_w[rows, :].T).astype(f16),
            "wo": np.ascontiguousarray(Wo_w[:, rows].T).astype(f16),
            "bq": Wq_b[rows].astype(np.float32).reshape(2, 128, 1),
            "bk": Wk_b[rows].astype(np.float32).reshape(2, 128, 1),
            "bv": Wv_b[rows].astype(np.float32).reshape(HPC, 64, 1),
        })
    return in_maps


def kernel(q, k, v, Wq_w, Wq_b, Wk_w, Wk_b, Wv_w, Wv_b, Wo_w, Wo_b):
    global _PROG
    args = [np.asarray(x, dtype=np.float32) for x in
            (q, k, v, Wq_w, Wq_b, Wk_w, Wk_b, Wv_w, Wv_b, Wo_w, Wo_b)]
    if _PROG is None:
        _PROG = _build()
    in_maps = _prepare(*args)
    res = run_bass_kernel_spmd(_PROG, in_maps, core_ids=list(range(N_CORES)))
    kernel.last_results = res
    Wo_b32 = args[10]
    out = np.empty((B, S, D), dtype=np.float32)
    for b in range(B):
        acc = res.results[GROUPS * b]["out"].astype(np.float32)
        for g in range(1, GROUPS):
            acc += res.results[GROUPS * b + g]["out"]
        out[b] = acc + Wo_b32
    return out


# revision 35
# speedup vs baseline: 15523.1320x; 1.0200x over previous
"""Multi-head attention (RoPE-full-dmodel variant) on 8 TRN2 NeuronCores.

Sharding: core c = (batch c//4, head-group c%4 of 4 heads).
 - W_q/W_k/W_v split column-wise by head (each core projects its 256 channels)
 - W_o split row-wise; per-core partial outputs summed on host (all-reduce at gather)

v1 restructure vs baseline (394us):
 - V projected directly seq-major (vT-tile stationary, wv moving): kills the
   32 PE transposes + chan-major staging copies; vT streamed per seq-tile
 - RoPE split into s-halves so K/Q projections (and the exp chain) start
   ~2x earlier; DVE/GpSimd column split rebalanced 896/128 (measured rates)
 - DMA emission ordered by first use: tables+kT in rope-pair order, then
   weights, qT, vT-per-tile, wo
 - softmax denominators: reciprocal_approx_fast straight from PSUM (replaces
   52us of [1,512] iterative reciprocals + staging copies)
 - kt/qt/ut/v split into per-block tiles so dependency tracking can't
   serialize attention on whole-tensor writes
"""
import os
import sys

for _p in ("/opt/trn_rl_repo", "/root/.axon_site/_ro/trn_rl_repo"):
    if os.path.isdir(_p) and _p not in sys.path:
        sys.path.insert(0, _p)

import numpy as np

import concourse.bacc as bacc
import concourse.tile as tile
import concourse.mybir as mybir
from concourse.bass_utils import run_bass_kernel_spmd

B, S, D = 2, 2048, 1024
H_TOT, DK = 16, 64
N_CORES, GROUPS = 8, 4
CH = 256            # channels (heads*dk) per core
KC = D // 128       # 8 d-model chunks
QB = S // 512       # 4 query blocks
ST = S // 128       # 16 seq tiles
HPC = 4             # heads per core
BASE = 10000.0

MM = mybir.dt.float16
F32 = mybir.dt.float32
AF = mybir.ActivationFunctionType

DVE_COLS = 896      # of each 1024-col half; rest on GpSimd (174 vs 30 Gel/s)

# even/odd permutation of the d_model axis: row r <- old d = 2r (r<512), 2(r-512)+1
_PERM = np.concatenate([np.arange(0, D, 2), np.arange(1, D, 2)])

_PROG = None


def _build():
    nc = bacc.Bacc("TRN2", target_bir_lowering=False, debug=False)
    qT = nc.dram_tensor("qT", (D, S), MM, kind="ExternalInput").ap()
    kT = nc.dram_tensor("kT", (D, S), MM, kind="ExternalInput").ap()
    vT = nc.dram_tensor("vT", (D, S), MM, kind="ExternalInput").ap()
    cosc = nc.dram_tensor("cosc", (D // 2, S), MM, kind="ExternalInput").ap()
    sinc = nc.dram_tensor("sinc", (D // 2, S), MM, kind="ExternalInput").ap()
    wq = nc.dram_tensor("wq", (D, CH), MM, kind="ExternalInput").ap()
    wk = nc.dram_tensor("wk", (D, CH), MM, kind="ExternalInput").ap()
    wv = nc.dram_tensor("wv", (D, CH), MM, kind="ExternalInput").ap()
    wo = nc.dram_tensor("wo", (CH, D), MM, kind="ExternalInput").ap()
    bq = nc.dram_tensor("bq", (2, 128, 1), F32, kind="ExternalInput").ap()
    bk = nc.dram_tensor("bk", (2, 128, 1), F32, kind="ExternalInput").ap()
    bv = nc.dram_tensor("bv", (HPC, 64, 1), F32, kind="ExternalInput").ap()
    out = nc.dram_tensor("out", (S, D), MM, kind="ExternalOutput").ap()
    dbg = None
    if os.environ.get("KERNEL_DEBUG"):
        dbg = {n: nc.dram_tensor(f"dbg_{n}", shp, MM, kind="ExternalOutput").ap()
               for n, shp in [("kt", (128, 512)), ("qt", (128, 1024)),
                              ("v", (128, 260)), ("ut", (128, 512))]}
        dbg["uraw"] = nc.dram_tensor("dbg_uraw", (65, 512), F32,
                                     kind="ExternalOutput").ap()
        dbg["bc"] = nc.dram_tensor("dbg_bc", (64, 512), F32,
                                   kind="ExternalOutput").ap()

    with tile.TileContext(nc) as tc:
      with (
          tc.tile_pool(name="consts", bufs=1) as consts,
          tc.tile_pool(name="qkv", bufs=1) as qkv,
          tc.tile_pool(name="expp", bufs=8) as expp,
          tc.tile_pool(name="vst", bufs=4) as vst,
          tc.tile_pool(name="misc", bufs=2) as misc,
          tc.tile_pool(name="outst", bufs=2) as outst,
          tc.tile_pool(name="ps_mix", bufs=4, space="PSUM") as ps_mix,
          tc.tile_pool(name="ps_sc", bufs=2, space="PSUM") as ps_sc,
      ):
        # persistent activation tiles, split per block for fine-grained deps
        v_t = [qkv.tile([128, HPC * 65], MM, tag=f"v{t}", name=f"v{t}")
               for t in range(ST)]
        for t in range(ST):
            nc.vector.memset(
                v_t[t][:].rearrange("p (h j) -> p h j", j=65)[:, :, 64], 1.0)
        kt_t = [[qkv.tile([128, 512], MM, tag=f"kt{c}_{b}", name=f"kt{c}_{b}")
                 for b in range(QB)] for c in range(2)]
        qt_t = [[qkv.tile([128, 1024], MM, tag=f"qt{c}_{p}", name=f"qt{c}_{p}")
                 for p in range(2)] for c in range(2)]
        ut_t = [[qkv.tile([128, 512], MM, tag=f"ut{c}_{b}", name=f"ut{c}_{b}")
                 for b in range(QB)] for c in range(2)]

        wq_sb = consts.tile([128, KC * CH], MM, tag="wq")
        wk_sb = consts.tile([128, KC * CH], MM, tag="wk")
        wv_sb = consts.tile([128, KC * CH], MM, tag="wv")
        wo_sb = consts.tile([128, 2 * D], MM, tag="wo")
        bq_sb, bk_sb, bv_sb = [], [], []

        with tc.tile_pool(name="p1", bufs=1) as p1:
            # ---------- DMA emission: order = service order ----------
            # chunk-group transfers: one 1MB gathered DMA per (tensor, half,
            # 4-chunk group), ordered to match the rope schedule k0,k1,q0,q1
            cos_g, sin_g = [None, None], [None, None]
            xk_g = [[None, None] for _ in range(2)]   # [half][group]
            xq_g = [[None, None] for _ in range(2)]

            def _gld(name, src, rows, half, tag):
                lo = 1024 * half
                x_ = p1.tile([128, 4096], MM, tag=tag, name=tag)
                s = src[rows:rows + 512, lo:lo + 1024].rearrange(
                    "(c p) n -> p c n", p=128)
                nc.sync.dma_start(x_[:].rearrange("p (c n) -> p c n", n=1024), s)
                return x_

            cos_g[0] = _gld("c", cosc, 0, 0, "cos_0")
            sin_g[0] = _gld("s", sinc, 0, 0, "sin_0")
            xk_g[0][0] = _gld("k", kT, 0, 0, "xk_0_0")
            xk_g[0][1] = _gld("k", kT, 512, 0, "xk_0_1")
            for c in range(KC):
                nc.sync.dma_start(wk_sb[:, CH * c:CH * (c + 1)],
                                  wk[128 * c:128 * (c + 1), :])
            cos_g[1] = _gld("c", cosc, 0, 1, "cos_1")
            sin_g[1] = _gld("s", sinc, 0, 1, "sin_1")
            xk_g[1][0] = _gld("k", kT, 0, 1, "xk_1_0")
            xk_g[1][1] = _gld("k", kT, 512, 1, "xk_1_1")
            xq_g[0][0] = _gld("q", qT, 0, 0, "xq_0_0")
            xq_g[0][1] = _gld("q", qT, 512, 0, "xq_0_1")
            for c in range(KC):
                nc.sync.dma_start(wq_sb[:, CH * c:CH * (c + 1)],
                                  wq[128 * c:128 * (c + 1), :])
            for c in range(2):
                t_ = consts.tile([128, 1], F32, tag=f"bq{c}", name=f"bq{c}")
                nc.sync.dma_start(t_[:], bq[c])
                bq_sb.append(t_)
                t_ = consts.tile([128, 1], F32, tag=f"bk{c}", name=f"bk{c}")
                nc.sync.dma_start(t_[:], bk[c])
                bk_sb.append(t_)
            for h in range(HPC):
                t_ = consts.tile([64, 1], F32, tag=f"bv{h}", name=f"bv{h}")
                nc.sync.dma_start(t_[:], bv[h])
                bv_sb.append(t_)
            for c in range(KC):
                nc.sync.dma_start(wv_sb[:, CH * c:CH * (c + 1)],
                                  wv[128 * c:128 * (c + 1), :])
            # vT per seq-tile-pair: gathered DMA, consumed streaming by the
            # inline vproj (tiles in outer pool: p1 closes before attention)
            xvt2 = []
            for tp in range(ST // 2):
                x_ = vst.tile([128, 2048], MM, tag="xvt", name=f"xvt{tp}")
                src = vT[:, 256 * tp:256 * (tp + 1)].rearrange(
                    "(c p) n -> p c n", p=128)
                nc.sync.dma_start(x_[:].rearrange("p (c n) -> p c n", n=256), src)
                xvt2.append(x_)
            xq_g[1][0] = _gld("q", qT, 0, 1, "xq_1_0")
            xq_g[1][1] = _gld("q", qT, 512, 1, "xq_1_1")
            for c in range(2):
                nc.sync.dma_start(wo_sb[:, D * c:D * (c + 1)],
                                  wo[128 * c:128 * (c + 1), :])

            # ---------- rope (per s-half, DVE only) ----------
            def _rope_half(xg, half):
                cs = lambda t: cos_g[half][:, 1024 * t:1024 * (t + 1)]
                ss = lambda t: sin_g[half][:, 1024 * t:1024 * (t + 1)]
                xx = lambda a: xg[half][a // 4][:, 1024 * (a % 4):1024 * (a % 4 + 1)]
                roped = [None] * KC
                for a in (0, 1, 4, 5):
                    b_ = a + 2
                    ca, cb = a % 4, b_ % 4
                    xa, xb = xx(a), xx(b_)
                    t1 = p1.tile([128, 1024], MM, tag="tmp", name="t1", bufs=3)
                    nc.vector.tensor_mul(t1[:], xa, cs(ca))
                    t2 = p1.tile([128, 1024], MM, tag="tmp", name="t2", bufs=3)
                    nc.vector.tensor_mul(t2[:], xb, ss(ca))
                    ra = p1.tile([128, 1024], MM, tag="roped", name="ra", bufs=6)
                    nc.vector.tensor_sub(ra[:], t1[:], t2[:])
                    t3 = p1.tile([128, 1024], MM, tag="tmp", name="t3", bufs=3)
                    nc.vector.tensor_mul(t3[:], xb, cs(cb))
                    t4 = p1.tile([128, 1024], MM, tag="tmp", name="t4", bufs=3)
                    nc.vector.tensor_mul(t4[:], xa, ss(cb))
                    rb = p1.tile([128, 1024], MM, tag="roped", name="rb", bufs=6)
                    nc.vector.tensor_add(rb[:], t3[:], t4[:])
                    roped[a], roped[b_] = ra, rb
                return roped

            def _proj(roped, w_sb, b_sb, dst_fn, half, store_dve=False):
                for c in range(2):
                    for i in range(2):
                        psum = ps_mix.tile([128, 512], F32, tag="mix", name="psum")
                        for d in range(KC):
                            lhsT = w_sb[:, CH * d + 128 * c: CH * d + 128 * (c + 1)]
                            nc.tensor.matmul(psum[:], lhsT,
                                             roped[d][:, 512 * i:512 * (i + 1)],
                                             start=(d == 0), stop=(d == KC - 1))
                        if store_dve:
                            nc.vector.tensor_scalar_add(
                                dst_fn(c, 2 * half + i), psum[:], b_sb[c][:])
                        else:
                            nc.scalar.activation(dst_fn(c, 2 * half + i), psum[:],
                                                 AF.Identity, bias=b_sb[c][:])

            def _kdst(c, s):
                return kt_t[c][s][:]

            def _qdst(c, s):
                return qt_t[c][s // 2][:, 512 * (s % 2):512 * (s % 2 + 1)]

            rk0 = _rope_half(xk_g, 0)
            _proj(rk0, wk_sb, bk_sb, _kdst, 0)
            rk1 = _rope_half(xk_g, 1)
            _proj(rk1, wk_sb, bk_sb, _kdst, 1)
            rq0 = _rope_half(xq_g, 0)
            _proj(rq0, wq_sb, bq_sb, _qdst, 0)
            # q-h1 feeds only qbp1 (needed ~70us later): demote its priority so
            # its projections fill PE slack instead of stalling the exp chain
            with tc.high_priority(offset=-50000):
                rq1 = _rope_half(xq_g, 1)
                _proj(rq1, wq_sb, bq_sb, _qdst, 1, store_dve=True)

        # ---------- V projection, directly seq-major ----------
        def _vproj_t(t):
            pv = ps_mix.tile([128, 256], F32, tag="mix", name="pv")
            xt = xvt2[t // 2]
            for d in range(KC):
                lhsT = xt[:, 256 * d + 128 * (t % 2):256 * d + 128 * (t % 2) + 128]
                nc.tensor.matmul(pv[:], lhsT,
                                 wv_sb[:, CH * d:CH * (d + 1)],
                                 start=(d == 0), stop=(d == KC - 1))
            dst = v_t[t][:].rearrange("p (h j) -> p h j", j=65)[:, :, 0:64]
            nc.scalar.copy(dst, pv[:])

        # ---------- attention + output projection ----------
        # Heads are processed in pairs (A=2hp on partitions 0-63, B=2hp+1 on
        # 64-127): the two score matmuls go to disjoint PE row-groups and run
        # concurrently; one exp covers both heads at FD=1024. Query halves are
        # sequential so only 2 UT accumulators are live (leaves ps_mix slots
        # for vproj/outproj).
        for _vt in range(ST):
            _vproj_t(_vt)

        ones64 = consts.tile([1, 64], MM, tag="ones64")
        nc.vector.memset(ones64[:], 1.0)
        dbg_nrm = {}

        def _norm_copy(put):
            uraw = misc.tile([65, 512], F32, tag="uraw", name="uraw")
            nc.vector.tensor_copy(uraw[:], put[:])
            return uraw

        def _normalize(h, qb, uraw):
            # broadcast den across 64 partitions with a K=1 PE matmul
            # (gpsimd partition_broadcast thrashes Q7 IRAM library reloads)
            ct, po = h // 2, 64 * (h % 2)
            den16 = misc.tile([1, 512], MM, tag="den16", name="den16")
            nc.vector.tensor_copy(den16[:], uraw[64:65, :])
            bcd = ps_mix.tile([64, 512], F32, tag="mix", name="bcd")
            nc.tensor.matmul(bcd[:], ones64[:], den16[:], start=True, stop=True)
            bc = misc.tile([64, 512], F32, tag="bc", name="bc", bufs=1)
            nc.vector.reciprocal_approx_fast(bc[:], bcd[:])
            dst = ut_t[ct][qb][po:po + 64, :]
            nc.vector.tensor_mul(dst, uraw[0:64, :], bc[:])
            nc.vector.tensor_scalar_add(dst, dst, bv_sb[h][:])
            if dbg is not None and h == 0 and qb == 0:
                dbg_nrm["uraw"] = uraw
                dbg_nrm["bc"] = bc

        outproj_pending = []

        def _flush_outproj():
            while outproj_pending:
                _outproj_qb(outproj_pending.pop(0))

        def _attention(qbp):
            for half in range(2):
                qb = 2 * qbp + half
                for hp in range(2):
                    hA, hB, ct = 2 * hp, 2 * hp + 1, hp
                    qA = qt_t[ct][qbp][0:64, 512 * half:512 * (half + 1)]
                    qB = qt_t[ct][qbp][64:128, 512 * half:512 * (half + 1)]
                    putA = ps_mix.tile([65, 512], F32, tag="mix", name="putA")
                    putB = ps_mix.tile([65, 512], F32, tag="mix", name="putB")
                    for t in range(ST):
                        ktile = kt_t[ct][t // 4]
                        co = 128 * (t % 4)
                        psc = ps_sc.tile([128, 1024], F32, tag="sc", name="psc")
                        nc.tensor.matmul(psc[:, 0:512],
                                         ktile[0:64, co:co + 128], qA,
                                         start=True, stop=True)
                        nc.tensor.matmul(psc[:, 512:1024],
                                         ktile[64:128, co:co + 128], qB,
                                         start=True, stop=True)
                        e = expp.tile([128, 1024], MM, tag="e", name="e")
                        nc.scalar.activation(e[:], psc[:], AF.Exp, scale=0.125)
                        nc.tensor.matmul(putA[:], v_t[t][:, 65 * hA:65 * hA + 65],
                                         e[:, 0:512],
                                         start=(t == 0), stop=(t == ST - 1),
                                         skip_group_check=True)
                        nc.tensor.matmul(putB[:], v_t[t][:, 65 * hB:65 * hB + 65],
                                         e[:, 512:1024],
                                         start=(t == 0), stop=(t == ST - 1),
                                         skip_group_check=True)
                    urA = _norm_copy(putA)
                    urB = _norm_copy(putB)
                    _normalize(hA, qb, urA)
                    _normalize(hB, qb, urB)
                    if hp == 0:
                        # previous qb's output projection: emitted mid-half so
                        # its PE priority sits behind this half's first scores
                        _flush_outproj()
                outproj_pending.append(qb)

        def _outproj_qb(qb):
            for si in range(4):
                st = 4 * qb + si
                pos = [ps_mix.tile([128, 512], F32, tag="mix", name="po_")
                       for _ in range(2)]
                for cc in range(2):
                    lhsT = ut_t[cc][qb][:, 128 * si:128 * (si + 1)]
                    for nb in range(2):
                        nc.tensor.matmul(
                            pos[nb][:], lhsT,
                            wo_sb[:, D * cc + 512 * nb: D * cc + 512 * (nb + 1)],
                            start=(cc == 0), stop=(cc == 1))
                for nb in range(2):
                    stg = outst.tile([128, 512], MM, tag="stg", name="stg")
                    nc.vector.tensor_copy(stg[:], pos[nb][:])
                    nc.sync.dma_start(
                        out[128 * st:128 * (st + 1), 512 * nb:512 * (nb + 1)],
                        stg[:])

        _attention(0)
        _attention(1)
        _flush_outproj()

        if dbg is not None:
            nc.sync.dma_start(dbg["kt"], kt_t[0][0][:])
            nc.sync.dma_start(dbg["qt"], qt_t[0][0][:])
            nc.sync.dma_start(dbg["v"], v_t[0][:])
            nc.sync.dma_start(dbg["ut"], ut_t[0][0][:])
            nc.sync.dma_start(dbg["uraw"], dbg_nrm["uraw"][:])
            nc.sync.dma_start(dbg["bc"], dbg_nrm["bc"][:])

    nc.compile()
    return nc


def _prepare(q, k, v, Wq_w, Wq_b, Wk_w, Wk_b, Wv_w, Wv_b, Wo_w, Wo_b):
    f16 = np.float16
    pos = np.arange(1, S + 1, dtype=np.float32)
    theta = (BASE ** (-2.0 * np.arange(D // 2, dtype=np.float32) / D)).astype(np.float32)
    ang = theta[:, None] * pos[None, :]
    cosc = np.cos(ang).astype(f16)
    sinc = np.sin(ang).astype(f16)

    per_batch = []
    for b in range(B):
        per_batch.append((
            np.ascontiguousarray(q[b].T[_PERM]).astype(f16),
            np.ascontiguousarray(k[b].T[_PERM]).astype(f16),
            np.ascontiguousarray(v[b].T).astype(f16),
        ))
    in_maps = []
    for c in range(N_CORES):
        b, g = divmod(c, GROUPS)
        rows = slice(CH * g, CH * (g + 1))
        qTb, kTb, vTb = per_batch[b]
        in_maps.append({
            "qT": qTb, "kT": kTb, "vT": vTb, "cosc": cosc, "sinc": sinc,
            "wq": np.ascontiguousarray(Wq_w[rows, :].T[_PERM]).astype(f16),
            "wk": np.ascontiguousarray(Wk_w[rows, :].T[_PERM]).astype(f16),
            "wv": np.ascontiguousarray(Wv_w[rows, :].T).astype(f16),
            "wo": np.ascontiguousarray(Wo_w[:, rows].T).astype(f16),
            "bq": Wq_b[rows].astype(np.float32).reshape(2, 128, 1),
            "bk": Wk_b[rows].astype(np.float32).reshape(2, 128, 1),
            "bv": Wv_b[rows].astype(np.float32).reshape(HPC, 64, 1),
        })
    return in_maps


def kernel(q, k, v, Wq_w, Wq_b, Wk_w, Wk_b, Wv_w, Wv_b, Wo_w, Wo_b):
    global _PROG
    args = [np.asarray(x, dtype=np.float32) for x in
            (q, k, v, Wq_w, Wq_b, Wk_w, Wk_b, Wv_w, Wv_b, Wo_w, Wo_b)]
    if _PROG is None:
        _PROG = _build()
    in_maps = _prepare(*args)
    res = run_bass_kernel_spmd(_PROG, in_maps, core_ids=list(range(N_CORES)))
    kernel.last_results = res
    Wo_b32 = args[10]
    out = np.empty((B, S, D), dtype=np.float32)
    for b in range(B):
        acc = res.results[GROUPS * b]["out"].astype(np.float32)
        for g in range(1, GROUPS):
            acc += res.results[GROUPS * b + g]["out"]
        out[b] = acc + Wo_b32
    return out
